# revision 19
# baseline (speedup 1.0000x reference)
"""BitFeedForward (BitNet-style FFN) Trainium2 kernel — 8-core data parallel.

kernel(**inputs) takes the FULL unsharded inputs of
nn_BitFeedForward_25280177504455:
    x  [4, 2048, 2048] f32, w1 [8192, 2048], b1 [8192],
    w2 [2048, 8192], b2 [2048]
and returns the full [4, 2048, 2048] f32 output.

Sharding: data-parallel over tokens (1024 tokens/core).  The host ships
weights pre-transposed (w1.T / w2.T, a pure layout change) so each core
can stream them with large contiguous DMA descriptors, convert to
ternary sign form on the fly (sign -> bf16 in SBUF, no DRAM spill), and
use them directly as matmul operands.  weight_quant's global mean|w| is
computed from per-core shards and combined with a tiny 8-core AllReduce
(8 bytes), so the full |w| reduction is done once across the chip
instead of 8x redundantly.

On-device flow per core (1024 tokens):
  A. |w| partial sums over this core's weight shard -> AllReduce -> mw1, mw2
  B. x stats (rms + absmax) -> r1/c1/beta1, quantize, PE-transpose -> q1T
     (SBUF resident)
  C. L1: stream w1T in 512-col slabs, sign to bf16, matmul
     h[t,i] = gelu(beta1*(q1T.T @ w1s + rb1 x b1)); bn_stats/absmax per
     token accumulate for the second rms; h spilled bf16.
  D. finalize r2/beta2; rebuild q2 from h, PE-transpose -> q2T (SBUF
     resident, aliases q1T's space)
  E. L2: stream w2T in 128-row o-bands, sign, matmul
     out[o,t] = beta2[t]*(w2s.T @ q2T + b2 x rb2) -> out written [o,t];
     host transposes back.
"""
import functools

import numpy as np
import ml_dtypes

from contextlib import ExitStack

import concourse.bacc as bacc
import concourse.tile as tile
from concourse import mybir
from concourse.bass_utils import run_bass_kernel_spmd

F32 = mybir.dt.float32
BF16 = mybir.dt.bfloat16

EPS_RMS = 1e-6
EPS_Q = 1e-5
# v + C lands in [2^23, 2^24) where fp32 spacing is 1.0 -> RNE integer round
C_RND = float(1.5 * 2.0**23)
P = 128
AX = mybir.AxisListType
ALU = mybir.AluOpType
AF = mybir.ActivationFunctionType

NCORES = 8
B, S, DIM = 4, 2048, 2048
INNER = 8192
OUT = DIM
TOK = B * S // NCORES   # 1024 tokens per core
TT = TOK // P           # 8 token tiles
KD = DIM // P           # 16 contraction chunks for L1
KI = INNER // P         # 64 contraction chunks for L2
NE1 = 16                # L1 slabs (512 inner cols each)
SL1 = INNER // NE1      # 512
NB2 = 16                # L2 o-bands (128 out cols each)
BO = OUT // NB2         # 128
WSHE = 2 * INNER * DIM // NCORES  # weight-shard elements per core (4.19M)
WSHC = WSHE // (P * 2048)         # 16 chunks of [128, 2048]


def build():
    from concourse.tile_rust import add_dep_helper

    nc = bacc.Bacc("TRN2", enable_partition_id=False, num_devices=NCORES)

    x_d = nc.dram_tensor("x", [TOK, DIM], F32, kind="ExternalInput")
    w1t_d = nc.dram_tensor("w1t", [DIM, INNER], F32, kind="ExternalInput")
    w2t_d = nc.dram_tensor("w2t", [INNER, OUT], F32, kind="ExternalInput")
    b1_d = nc.dram_tensor("b1", [1, INNER], F32, kind="ExternalInput")
    b2_d = nc.dram_tensor("b2", [1, OUT], F32, kind="ExternalInput")
    wsh_d = nc.dram_tensor("wsh", [P, WSHE // P], F32, kind="ExternalInput")
    idf_d = nc.dram_tensor("identf", [P, P], F32, kind="ExternalInput")
    idb_d = nc.dram_tensor("identb", [P, P], BF16, kind="ExternalInput")
    ones_d = nc.dram_tensor("ones", [P, P], F32, kind="ExternalInput")
    out_d = nc.dram_tensor("out", [OUT, TOK], F32, kind="ExternalOutput")

    with ExitStack() as ctx:
        tc = ctx.enter_context(tile.TileContext(nc))
        pool = lambda name, bufs, space="SBUF": ctx.enter_context(
            tc.tile_pool(name=name, bufs=bufs, space=space))

        consts = pool("consts", 1)
        stag = pool("stag", 3)        # f32 staging: w1T/w2T slab tiles
        stagx = pool("stagx", 2)      # f32 staging: wsh/x tiles
        junkp = pool("junkp", 1)
        wring = pool("wring", 2)      # bf16 sign-weight slabs
        qTp = pool("qTp", 1)          # q1T then q2T (aliased)
        qp = pool("qp", 1)            # q1 bf16 chunks
        hcp = pool("hcp", 2)          # h / q2 bf16 tiles
        outp = pool("outp", 2)        # f32 drains
        vecs = pool("vecs", 2)
        bch = pool("bch", 2)
        bb = pool("bb", 1)
        pers = pool("pers", 1)
        dram = pool("dram", 1, "DRAM")
        ps_g = pool("ps_g", 4, "PSUM")
        ps_t = pool("ps_t", 2, "PSUM")
        ps_v = pool("ps_v", 2, "PSUM")

        identf = consts.tile([P, P], F32)
        identb = consts.tile([P, P], BF16)
        ones = consts.tile([P, P], F32)
        nc.sync.dma_start(identf, idf_d[:, :])
        nc.sync.dma_start(identb, idb_d[:, :])
        nc.sync.dma_start(ones, ones_d[:, :])

        h_dram = dram.tile([TOK, INNER], BF16)
        cc_in = dram.tile([1, 2], F32)
        cc_out = dram.tile([1, 2], F32, addr_space="Shared")

        state = {"pe": None}

        def pe(instr):
            if state["pe"] is not None:
                add_dep_helper(instr.ins, state["pe"].ins, sync=False,
                               reason="pe chain")
            state["pe"] = instr
            return instr

        # ---- persistent scalars/vectors ----
        wsums4 = pers.tile([P, WSHC * 4], F32, tag="wsums4")
        beta1s = pers.tile([P, TT], F32, tag="beta1s")
        r2s = pers.tile([P, TT], F32, tag="r2s")
        m2s = pers.tile([P, TT], F32, tag="m2s")
        rb1T = pers.tile([1, TOK], F32, tag="rb1T")
        rb2row = pers.tile([1, TOK], F32, tag="rb2row")
        beta2row = pers.tile([1, TOK], F32, tag="beta2row")
        mws = pers.tile([P, 2], F32, tag="mws")
        stv2 = pers.tile([P, TT, NE1, 6], F32, tag="stv2")

        # ================= A: weight-shard |w| sums + AllReduce ========
        # |w| partial sums ride the scalar engine (Abs + accum_out) so the
        # vector engine is free for the concurrent x-statistics.
        for j in range(WSHC * 4):
            wt = stagx.tile([P, 512], F32, tag="stagx")
            nc.sync.dma_start(wt, wsh_d[:, j * 512:(j + 1) * 512])
            jk = junkp.tile([P, 512], F32, tag="junk")
            nc.scalar.activation(jk, wt, AF.Abs,
                                 accum_out=wsums4[:, j:j + 1])
        cc_sb = vecs.tile([1, 2], F32, tag="cc_sb")
        for half in range(2):
            col = vecs.tile([P, 1], F32, tag="wcol")
            nc.vector.tensor_reduce(
                col, wsums4[:, half * (WSHC * 2):(half + 1) * (WSHC * 2)],
                axis=AX.X, op=ALU.add)
            pssc = ps_v.tile([1, 1], F32, tag="psv")
            pe(nc.tensor.matmul(pssc, col, ones[:, 0:1], start=True,
                                stop=True))
            nc.scalar.copy(cc_sb[0:1, half:half + 1], pssc)
        nc.sync.dma_start(cc_in, cc_sb)
        nc.gpsimd.collective_compute(
            "AllReduce", ALU.add,
            replica_groups=[list(range(NCORES))],
            ins=[cc_in.opt()], outs=[cc_out.opt()])
        cc_rb = vecs.tile([1, 2], F32, tag="cc_rb")
        nc.sync.dma_start(cc_rb, cc_out)
        for half in range(2):
            psbc = ps_v.tile([P, 1], F32, tag="psv")
            pe(nc.tensor.matmul(psbc, ones[0:1, :], cc_rb[0:1, half:half + 1],
                                start=True, stop=True))
            nc.scalar.mul(mws[:, half:half + 1], psbc,
                          1.0 / (INNER * DIM * 127.0))
        mw1 = mws[:, 0:1]
        mw2 = mws[:, 1:2]

        def finalize_scale(stv, M, WID, r_out):
            mv = vecs.tile([P, 2], F32, tag="bn_mv")
            nc.vector.bn_aggr(mv, stv)
            msq = vecs.tile([P, 1], F32, tag="msq")
            nc.vector.tensor_tensor(msq, mv[:, 0:1], mv[:, 0:1], op=ALU.mult)
            nc.vector.tensor_tensor(msq, msq, mv[:, 1:2], op=ALU.add)
            nc.vector.tensor_scalar_add(msq, msq, EPS_RMS)
            y = vecs.tile([P, 1], F32, tag="sq_y")
            nc.scalar.sqrt(y, msq)
            d_ = vecs.tile([P, 1], F32, tag="sq_d")
            nc.vector.reciprocal(d_, y)
            nc.vector.tensor_tensor(d_, msq, d_, op=ALU.mult)
            nc.vector.tensor_tensor(y, y, d_, op=ALU.add)
            nc.vector.tensor_scalar_mul(y, y, 0.5 * (float(WID) ** 0.5))
            a = vecs.tile([P, 1], F32, tag="a")
            nc.vector.reciprocal(a, y)
            c = vecs.tile([P, 1], F32, tag="c")
            nc.vector.tensor_tensor(c, a, M, op=ALU.mult)
            nc.vector.tensor_scalar_max(c, c, EPS_Q)
            r = vecs.tile([P, 1], F32, tag="r")
            nc.vector.reciprocal(r, c)
            nc.vector.tensor_tensor(r, r, a, op=ALU.mult)
            nc.vector.tensor_scalar_mul(r_out, r, 127.0)
            return c

        def col_to_row(col, row_slice):
            pst = ps_v.tile([1, P], F32, tag="psv")
            pe(nc.tensor.transpose(pst, col, identf))
            nc.scalar.copy(row_slice, pst)

        # ================= B: x-phase ==================================
        q1T = qTp.tile([P, KD, TOK], BF16, tag="qT", name="q1T")
        for tt in range(TT):
            stv = vecs.tile([P, 4, 6], F32, tag="stv1")
            M1 = vecs.tile([P, 1], F32, tag="M1")
            for cc in range(4):
                xt = stagx.tile([P, 512], F32, tag="stagx")
                nc.sync.dma_start(xt, x_d[tt * P:(tt + 1) * P,
                                          cc * 512:(cc + 1) * 512])
                nc.vector.bn_stats(stv[:, cc, :], xt)
                mx = vecs.tile([P, 1], F32, tag="mx")
                nc.vector.tensor_reduce(mx, xt, axis=AX.X, op=ALU.max,
                                        apply_absolute_value=True)
                if cc == 0:
                    nc.vector.tensor_copy(out=M1, in_=mx)
                else:
                    nc.vector.tensor_tensor(M1, M1, mx, op=ALU.max)
            r1 = vecs.tile([P, 1], F32, tag="r1")
            c1 = finalize_scale(stv, M1, DIM, r1)
            beta1 = beta1s[:, tt:tt + 1]
            nc.vector.tensor_tensor(beta1, c1, mw1, op=ALU.mult)
            rb1 = vecs.tile([P, 1], F32, tag="rb1")
            nc.vector.reciprocal(rb1, beta1)
            col_to_row(rb1, rb1T[0:1, tt * P:(tt + 1) * P])
            for cc in range(4):
                xt = stagx.tile([P, 512], F32, tag="stagx")
                nc.sync.dma_start(xt, x_d[tt * P:(tt + 1) * P,
                                          cc * 512:(cc + 1) * 512])
                xq = stagx.tile([P, 512], F32, tag="stagx")
                nc.vector.tensor_scalar(xq, xt, r1, C_RND, op0=ALU.mult,
                                        op1=ALU.add)
                q1 = qp.tile([P, 512], BF16, tag="q1")
                nc.vector.tensor_scalar(q1, xq, C_RND, None,
                                        op0=ALU.subtract)
                pst = ps_t.tile([P, 512], BF16, tag="pst")
                for j in range(4):
                    pe(nc.tensor.transpose(pst[:, j * P:(j + 1) * P],
                                           q1[:, j * P:(j + 1) * P], identb))
                nc.vector.tensor_copy(
                    out=q1T[:, 4 * cc:4 * (cc + 1), tt * P:(tt + 1) * P],
                    in_=pst.rearrange("p (a b) -> p a b", b=P))

        # ================= C: L1 slabs =================================
        for e in range(NE1):
            ws = wring.tile([P, KD, SL1], BF16, tag="w", name=f"ws1_{e}")
            for dc in range(KD):
                wt = stag.tile([P, SL1], F32, tag="stag")
                nc.sync.dma_start(
                    wt, w1t_d[dc * P:(dc + 1) * P, e * SL1:(e + 1) * SL1])
                nc.scalar.sign(ws[:, dc, :], wt)
            bc = bch.tile([1, SL1], F32, tag="bc")
            nc.sync.dma_start(bc, b1_d[0:1, e * SL1:(e + 1) * SL1])
            for tt in range(TT):
                pg = ps_g.tile([P, SL1], F32, tag="psg")
                pe(nc.tensor.matmul(pg, rb1T[0:1, tt * P:(tt + 1) * P], bc,
                                    start=True, stop=False))
                for dc in range(KD):
                    pe(nc.tensor.matmul(pg, q1T[:, dc, tt * P:(tt + 1) * P],
                                        ws[:, dc, :], start=False,
                                        stop=(dc == KD - 1)))
                hc = hcp.tile([P, SL1], BF16, tag="h")
                nc.scalar.activation(hc, pg, AF.Gelu,
                                     scale=beta1s[:, tt:tt + 1])
                nc.vector.bn_stats(stv2[:, tt, e, :], hc)
                mx = vecs.tile([P, 1], F32, tag="mx")
                nc.vector.tensor_reduce(mx, hc, axis=AX.X, op=ALU.max,
                                        apply_absolute_value=True)
                m2 = m2s[:, tt:tt + 1]
                if e == 0:
                    nc.vector.tensor_copy(out=m2, in_=mx)
                else:
                    nc.vector.tensor_tensor(m2, m2, mx, op=ALU.max)
                nc.sync.dma_start(
                    h_dram[tt * P:(tt + 1) * P, e * SL1:(e + 1) * SL1], hc)

        # ================= finalize L2 scales ==========================
        for tt in range(TT):
            r2 = r2s[:, tt:tt + 1]
            c2 = finalize_scale(stv2[:, tt, :, :], m2s[:, tt:tt + 1], INNER,
                                r2)
            beta2 = vecs.tile([P, 1], F32, tag="beta2")
            nc.vector.tensor_tensor(beta2, c2, mw2, op=ALU.mult)
            rb2 = vecs.tile([P, 1], F32, tag="rb2")
            nc.vector.reciprocal(rb2, beta2)
            col_to_row(rb2, rb2row[0:1, tt * P:(tt + 1) * P])
            col_to_row(beta2, beta2row[0:1, tt * P:(tt + 1) * P])
        bb0 = bb.tile([P, 512], F32, tag="bb0")
        bb1 = bb.tile([P, 512], F32, tag="bb1")
        nc.gpsimd.partition_broadcast(bb0, beta2row[0:1, 0:512])
        nc.gpsimd.partition_broadcast(bb1, beta2row[0:1, 512:1024])
        bbs = [bb0, bb1]

        # ================= D: rebuild q2, transpose -> q2T =============
        q2T = qTp.tile([P, KI, TOK], BF16, tag="qT", name="q2T")
        for tt in range(TT):
            for ic in range(INNER // 512):
                hr = hcp.tile([P, 512], BF16, tag="h")
                nc.sync.dma_start(
                    hr, h_dram[tt * P:(tt + 1) * P, ic * 512:(ic + 1) * 512])
                hq = stagx.tile([P, 512], F32, tag="stagx")
                nc.vector.tensor_scalar(hq, hr, r2s[:, tt:tt + 1], C_RND,
                                        op0=ALU.mult, op1=ALU.add)
                q2c = hcp.tile([P, 512], BF16, tag="q2c")
                nc.vector.tensor_scalar(q2c, hq, C_RND, None,
                                        op0=ALU.subtract)
                pst = ps_t.tile([P, 512], BF16, tag="pst")
                for j in range(4):
                    pe(nc.tensor.transpose(pst[:, j * P:(j + 1) * P],
                                           q2c[:, j * P:(j + 1) * P], identb))
                nc.scalar.copy(
                    q2T[:, 4 * ic:4 * (ic + 1), tt * P:(tt + 1) * P],
                    pst.rearrange("p (a b) -> p a b", b=P))

        # ================= E: L2 o-bands ===============================
        for b in range(NB2):
            ws2 = wring.tile([P, KI, BO], BF16, tag="w", name=f"ws2_{b}")
            for g in range(16):
                wt = stag.tile([P, 4, BO], F32, tag="stag")
                nc.sync.dma_start(
                    wt,
                    w2t_d[g * 512:(g + 1) * 512,
                          b * BO:(b + 1) * BO].rearrange(
                              "(k p) o -> p k o", p=P))
                nc.scalar.sign(ws2[:, 4 * g:4 * (g + 1), :], wt)
            bc2 = bch.tile([1, BO], F32, tag="bc")
            nc.sync.dma_start(bc2, b2_d[0:1, b * BO:(b + 1) * BO])
            for tg in range(2):
                pb = ps_g.tile([P, 512], F32, tag="psg")
                pe(nc.tensor.matmul(pb, bc2,
                                    rb2row[0:1, tg * 512:(tg + 1) * 512],
                                    start=True, stop=False))
                for kc in range(KI):
                    pe(nc.tensor.matmul(pb, ws2[:, kc, :],
                                        q2T[:, kc, tg * 512:(tg + 1) * 512],
                                        start=False, stop=(kc == KI - 1)))
                ob = outp.tile([P, 512], F32, tag="ob")
                nc.vector.tensor_tensor(ob, pb, bbs[tg], op=ALU.mult)
                nc.sync.dma_start(
                    out_d[b * BO:(b + 1) * BO, tg * 512:(tg + 1) * 512], ob)

    nc.compile()
    return nc


@functools.lru_cache(maxsize=1)
def _get_nc():
    return build()


def kernel(x, w1, b1, w2, b2, _trace=False):
    nc = _get_nc()
    xf = np.ascontiguousarray(x.reshape(B * S, DIM), dtype=np.float32)
    w1 = np.asarray(w1, dtype=np.float32)
    w2 = np.asarray(w2, dtype=np.float32)
    w1f = w1.reshape(-1)
    w2f = w2.reshape(-1)
    shard = w1f.size // NCORES
    common = {
        "w1t": np.ascontiguousarray(w1.T),
        "w2t": np.ascontiguousarray(w2.T),
        "b1": np.ascontiguousarray(b1, dtype=np.float32).reshape(1, INNER),
        "b2": np.ascontiguousarray(b2, dtype=np.float32).reshape(1, OUT),
        "identf": np.eye(P, dtype=np.float32),
        "identb": np.eye(P, dtype=np.float32).astype(ml_dtypes.bfloat16),
        "ones": np.ones((P, P), dtype=np.float32),
    }
    in_maps = []
    for c in range(NCORES):
        wsh = np.concatenate([
            w1f[c * shard:(c + 1) * shard].reshape(P, -1),
            w2f[c * shard:(c + 1) * shard].reshape(P, -1)], axis=1)
        in_maps.append({
            "x": xf[c * TOK:(c + 1) * TOK],
            "wsh": np.ascontiguousarray(wsh),
            **common,
        })
    res = run_bass_kernel_spmd(nc, in_maps, core_ids=list(range(NCORES)),
                               trace=_trace)
    out = np.concatenate(
        [res.results[c]["out"].T for c in range(NCORES)], axis=0)
    out = out.reshape(B, S, DIM)
    if _trace:
        return out, res
    return out
